# revision 20
# baseline (speedup 1.0000x reference)
"""GATv2 (2-layer) Trainium2 Bass kernel, 8-core SPMD.

Strategy (node-partitioned):
- Core k owns destination nodes [k*6250, (k+1)*6250). Host bins each core's
  edges by dst, sorts by dst, groups them into 64-node destination windows.
- Layer 1 node transforms (x_l1/x_r1) are precomputed on the host and
  shipped as a p-major global bf16 table + per-core x_r slice, skipping the
  first on-device node matmul and AllGather. Layer 2 computes x_l2/x_r2 on
  the PE from h1 and AllGathers the x_l2 table (bf16) into DRAM.
- Edge phase: x_j = x_l[src] rows (256B) are fetched with gpsimd dma_gather
  (int16 indices; the table is split at row 32768 into A/B regions, each
  window's edges grouped by region and padded to whole 128-edge tiles).
- x_r[dst] is NOT gathered: per tile the PE expands it as z = sTp^T @
  xr_win, where sTp is a host-built parity-padded one-hot S^T [128n, 128e]
  and xr_win the resident SBUF x_r column. Scalar drains the PSUM groups so
  the PE free-runs ahead of the gather stream; ev = z + x_j on DVE.
- alpha = sum_c leakyrelu(ev, 0.2)*att via DVE mult+reduce; exp(alpha) is
  written by the scalar engine directly into the segsum rhs columns 128:130
  (unnormalized weights; the segment-softmax max subtraction cancels).
- Segment sum on PE: per tile a host-built one-hot S [128e, 64n] is
  stationary; rhs = [w*x_j | w] (130 cols). PSUM accumulates across a
  window's tiles; dumps land in a node-major SBUF accumulator.
- Finalize: h = acc[:, :128]/(denom+1e-16) + bias (+leaky 0.01 for layer
  1). Layer 2 reuses the identical edge structure. Output = h1 + h2 of the
  owned slice; host reassembles cores.
"""
import os
import numpy as np
import ml_dtypes

BF = ml_dtypes.bfloat16

N = 50000
E = 640000
HID = 128
HEADS = 2
C = 64
ATT_SLOPE = 0.2
OUT_SLOPE = 0.01
NCORES = 8
TILE = 128
WIN = 128
SIM_LEAKY = bool(int(os.environ.get("GAT_SIM_LEAKY", "0")))


def configure(n=50000, e=640000, chunk_tiles=20, split=32768):
    global N, E, NPC, NPC_PAD, WINS, NTN, NP_GLOB, SPLIT, CHUNK_TILES
    N, E = n, e
    NPC = N // NCORES
    NPC_PAD = ((NPC + TILE - 1) // TILE) * TILE
    WINS = NPC_PAD // WIN
    NTN = NPC_PAD // TILE
    NP_GLOB = NPC_PAD * NCORES
    SPLIT = min(split, NP_GLOB)
    CHUNK_TILES = chunk_tiles


configure()


def _pack_idx16(idx):
    n = len(idx)
    cols = -(-n // 16)
    buf = np.zeros((cols, 16), dtype=np.int16)
    buf.reshape(-1)[:n] = idx.astype(np.int16)
    return np.tile(buf.T, (8, 1))


def _table_row(node_pad_global):
    """Row of a padded-global node in the p-major gather table."""
    k = node_pad_global // NPC_PAD
    loc = node_pad_global % NPC_PAD
    return k * NPC_PAD + (loc % 128) * NTN + loc // 128


def prep_edges(edge_index):
    src = np.asarray(edge_index[0], dtype=np.int64)
    dst = np.asarray(edge_index[1], dtype=np.int64)
    core_of = dst // NPC
    gp_all = (src // NPC) * NPC_PAD + (src % NPC)
    row_all = _table_row(gp_all)

    cores = []
    cnt = np.zeros((NCORES, WINS, 2), dtype=np.int64)
    for k in range(NCORES):
        m = core_of == k
        rj = row_all[m]
        dl = dst[m] - k * NPC
        order = np.argsort(dl, kind="stable")
        rj, dl = rj[order], dl[order]
        w = dl // WIN
        a = rj < SPLIT
        cnt[k, :, 0] = np.bincount(w[a], minlength=WINS)
        cnt[k, :, 1] = np.bincount(w[~a], minlength=WINS)
        cores.append((rj, dl, w, a))

    budget = (-(-cnt // TILE)).max(axis=0)      # [WINS, 2]
    tile_win, tile_part = [], []
    for part in (0, 1):
        for wi in range(WINS):
            tile_win += [wi] * int(budget[wi, part])
            tile_part += [part] * int(budget[wi, part])
    t_tot = len(tile_win)
    t_a = int(budget[:, 0].sum())

    per_core = []
    for k in range(NCORES):
        rj, dl, w, a = cores[k]
        idx_j = np.zeros(t_tot * TILE, dtype=np.int16)
        s_col = np.full(t_tot * TILE, -1, dtype=np.int64)
        pos = 0
        for part in (0, 1):
            mp = a if part == 0 else ~a
            for wi in range(WINS):
                sel = mp & (w == wi)
                r_sel, d_sel = rj[sel], dl[sel]
                n_real = len(r_sel)
                n_slot = int(budget[wi, part]) * TILE
                idx_j[pos:pos + n_real] = (r_sel - (SPLIT if part else 0)).astype(np.int16)
                s_col[pos:pos + n_real] = d_sel - wi * WIN
                pos += n_slot
        assert pos == t_tot * TILE

        s_mat = np.zeros((t_tot * TILE, WIN), dtype=BF)
        real = s_col >= 0
        s_mat[np.nonzero(real)[0], s_col[real]] = 1.0
        s_mat = s_mat.reshape(t_tot, TILE, WIN).transpose(1, 0, 2)

        # S^T for the PE x_r expansion: window == node tile (WIN=128), so
        # sTp[n_local, e] = 1 and rhs is the full xr_sb column wi
        sT = np.zeros((t_tot * TILE, 128), dtype=BF)
        sT[np.nonzero(real)[0], s_col[real].astype(np.int64)] = 1.0
        # lhsT layout: [128 n partitions, t, 128 e]
        sT = sT.reshape(t_tot, TILE, 128).transpose(2, 0, 1)

        per_core.append({
            "idxJ": _pack_idx16(idx_j),
            "Smat": np.ascontiguousarray(s_mat),
            "sTp": np.ascontiguousarray(sT),
        })

    meta = {"t_tot": t_tot, "t_a": t_a,
            "tile_win": tile_win, "tile_part": tile_part}
    return meta, per_core


def build_bass(meta):
    from concourse import bacc, mybir, tile

    F32, BF16, I16 = mybir.dt.float32, mybir.dt.bfloat16, mybir.dt.int16
    AF = mybir.ActivationFunctionType
    ALU = mybir.AluOpType

    t_tot, t_a = meta["t_tot"], meta["t_a"]
    tile_win, tile_part = meta["tile_win"], meta["tile_part"]
    n_chunks = -(-t_tot // CHUNK_TILES)

    nc = bacc.Bacc("TRN2", target_bir_lowering=False, debug=False,
                   num_devices=NCORES)

    xl1_d = nc.dram_tensor("xl1tab", [NP_GLOB, HID], BF16,
                           kind="ExternalInput")
    xr1_d = nc.dram_tensor("xr1sb", [128, NTN * HID], BF16,
                           kind="ExternalInput")
    w2_d = nc.dram_tensor("w2T", [HID, 2 * HID], BF16, kind="ExternalInput")
    att1_d = nc.dram_tensor("attbc1", [128, HID], BF16, kind="ExternalInput")
    att2_d = nc.dram_tensor("attbc2", [128, HID], BF16, kind="ExternalInput")
    b1_d = nc.dram_tensor("bias1", [128, HID], F32, kind="ExternalInput")
    b2_d = nc.dram_tensor("bias2", [128, HID], F32, kind="ExternalInput")
    id_d = nc.dram_tensor("ident", [128, 128], BF16, kind="ExternalInput")
    idxj_d = nc.dram_tensor("idxJ", [128, t_tot * 8], I16, kind="ExternalInput")
    smat_d = nc.dram_tensor("Smat", [128, t_tot, WIN], BF16, kind="ExternalInput")
    stp_d = nc.dram_tensor("sTp", [128, t_tot, 128], BF16, kind="ExternalInput")
    out_d = nc.dram_tensor("out", [128, NTN, HID], F32, kind="ExternalOutput")

    with tile.TileContext(nc) as tc:
        with (
            tc.tile_pool(name="const", bufs=1) as cpool,
            tc.tile_pool(name="node", bufs=1) as npool,
            tc.tile_pool(name="edge", bufs=2) as epool,
            tc.tile_pool(name="mmps", bufs=2, space="PSUM") as mmps,
            tc.tile_pool(name="zps", bufs=2, space="PSUM") as zpool,
            tc.tile_pool(name="wps", bufs=2, space="PSUM") as wps,
            tc.tile_pool(name="dram", bufs=1, space="DRAM") as dpool,
        ):
            w2_t = cpool.tile([HID, 2 * HID], BF16, tag="w2")
            att1_t = cpool.tile([128, HID], BF16, tag="att1")
            att2_t = cpool.tile([128, HID], BF16, tag="att2")
            b1_t = cpool.tile([128, HID], F32, tag="b1")
            b2_t = cpool.tile([128, HID], F32, tag="b2")
            id_t = cpool.tile([128, 128], BF16, tag="ident")

            for tdst, tsrc in ((w2_t, w2_d), (att1_t, att1_d),
                               (att2_t, att2_d), (b1_t, b1_d), (b2_t, b2_d),
                               (id_t, id_d)):
                nc.sync.dma_start(tdst[:], tsrc[:])

            h1_t = npool.tile([128, NTN, HID], BF16, tag="h1")
            h1T_t = npool.tile([HID, NPC_PAD], BF16, tag="h1T")
            acc_t = npool.tile([128, WINS, HID + 2], F32, tag="acc")

            cin = dpool.tile([128, NPC_PAD], BF16, tag="cin")
            xl_full = dpool.tile([NP_GLOB, HID], BF16, tag="xlfull",
                                 addr_space="Shared")

            def leaky(dst_ap, src_ap, slope, tag, pool=None):
                # scalar-engine Lrelu ignores alpha on HW; use max(x, a*x)
                tmp = (pool or epool).tile(list(src_ap.shape), src_ap.dtype,
                                           tag=tag)
                nc.scalar.mul(tmp[:], src_ap, slope)
                nc.vector.tensor_tensor(out=dst_ap, in0=src_ap,
                                        in1=tmp[:], op=ALU.max)

            def finalize(li, lo, hi, h_out, b_t, apply_leaky,
                         bias_full=None):
                """Normalize acc windows [lo, hi) into h_out columns."""
                nw = hi - lo
                den = npool.tile([128, nw, 2], F32, tag="den",
                                 name=f"den_{li}_{lo}")
                nc.vector.tensor_scalar_add(den[:], acc_t[:, lo:hi, HID:],
                                            1e-16)
                rden = npool.tile([128, nw, 2], F32, tag="rden",
                                  name=f"rden_{li}_{lo}")
                nc.vector.reciprocal(rden[:], den[:])
                nc.vector.tensor_tensor(
                    out=h_out[:, lo:hi, :].rearrange("p t (h c) -> p t h c",
                                                     h=2),
                    in0=acc_t[:, lo:hi, :HID].rearrange(
                        "p t (h c) -> p t h c", h=2),
                    in1=rden[:].unsqueeze(3).to_broadcast([128, nw, 2, C]),
                    op=ALU.mult)
                nc.vector.tensor_tensor(
                    out=h_out[:, lo:hi, :], in0=h_out[:, lo:hi, :],
                    in1=(bias_full[:, lo:hi, :] if bias_full is not None else
                         b_t[:].unsqueeze(1).to_broadcast([128, nw, HID])),
                    op=ALU.add)
                if apply_leaky:
                    leaky(h_out[:, lo:hi, :], h_out[:, lo:hi, :], OUT_SLOPE,
                          "lkf", pool=npool)

            xl_sb2 = npool.tile([128, NTN, HID], BF16, tag="xlsb")
            xr_sb2 = npool.tile([128, NTN, HID], BF16, tag="xrsb2")

            def l1_post(lo, hi):
                """After h1 windows [lo, hi) close: finalize them, build
                h1T tiles, run layer-2 node matmuls, write the cin slice."""
                finalize(0, lo, hi, h1_t, b1_t, True)
                for t in range(lo, hi):
                    tp = mmps.tile([128, 128], BF16, tag="tps")
                    nc.tensor.transpose(tp[:], h1_t[:, t, :], id_t[:])
                    nc.scalar.copy(h1T_t[:, t * 128:(t + 1) * 128], tp[:])
                    ps = mmps.tile([128, 2 * HID], F32, tag="nodeps")
                    nc.tensor.matmul(ps[:], h1T_t[:, t * 128:(t + 1) * 128],
                                     w2_t[:], start=True, stop=True)
                    nc.scalar.copy(xl_sb2[:, t, :], ps[:, :HID])
                    nc.scalar.copy(xr_sb2[:, t, :], ps[:, HID:])
                nc.sync.dma_start(
                    cin[:, lo * HID:hi * HID],
                    xl_sb2[:, lo:hi, :].rearrange("p t f -> p (t f)"))
                # pre-add b2 into h1 (h1T already extracted above); layer-2
                # finalize then adds h1+b2 in one pass, killing the tail add
                nc.vector.tensor_tensor(
                    out=h1_t[:, lo:hi, :], in0=h1_t[:, lo:hi, :],
                    in1=b2_t[:].unsqueeze(1).to_broadcast(
                        [128, hi - lo, HID]),
                    op=ALU.add)

            def layer(li, att_t, b_t, h_out, xr_sb, xl_tab, hooks=None):
                nc.vector.memset(acc_t[:], 0.0)
                cur = None  # (win, part, psum_tile)
                for ci in range(n_chunks):
                    t0 = ci * CHUNK_TILES
                    t1 = min(t0 + CHUNK_TILES, t_tot)
                    nt = t1 - t0

                    xj = epool.tile([128, CHUNK_TILES, HID], BF16, tag="xj",
                                    bufs=3)
                    s_t = epool.tile([128, CHUNK_TILES, WIN], BF16, tag="smat")
                    nc.sync.dma_start(s_t[:, :nt, :], smat_d[:, t0:t1, :])
                    stp_t = epool.tile([128, CHUNK_TILES, 128], BF16,
                                       tag="stpc")
                    nc.sync.dma_start(stp_t[:, :nt, :], stp_d[:, t0:t1, :])
                    idxj_t = epool.tile([128, CHUNK_TILES * 8], I16,
                                        tag="idxjc", bufs=3)
                    nc.sync.dma_start(idxj_t[:, :nt * 8], idxj_d[:, t0 * 8:t1 * 8])
                    spans = []
                    if t0 < t_a:
                        spans.append((t0, min(t1, t_a), 0))
                    if t1 > t_a:
                        spans.append((max(t0, t_a), t1, 1))
                    for (sa, sb_, part) in spans:
                        n_i = (sb_ - sa) * TILE
                        tab = xl_tab[SPLIT:NP_GLOB, :] if part else \
                            xl_tab[0:SPLIT, :]
                        nc.gpsimd.dma_gather(
                            out_ap=xj[:, sa - t0:sb_ - t0, :], in_ap=tab,
                            idxs_ap=idxj_t[:, (sa - t0) * 8:(sb_ - t0) * 8],
                            num_idxs=n_i, num_idxs_reg=n_i, elem_size=HID,
                            single_packet=False)

                    # x_r[dst] expansion on the PE: z = sTp^T @ xr_win.
                    # Scalar drains each PSUM group to SBUF immediately so the
                    # PE free-runs ahead of the gather; ev = z + xj on DVE.
                    z_sb = epool.tile([128, CHUNK_TILES, HID], BF16, tag="zsb")
                    ev = epool.tile([128, CHUNK_TILES, HID], BF16, tag="ev")
                    for g0 in range(0, nt, 4):
                        g1 = min(g0 + 4, nt)
                        zps = zpool.tile([128, 4, HID], F32, tag="zps")
                        for ti in range(g0, g1):
                            wi = tile_win[t0 + ti]
                            nc.tensor.matmul(
                                zps[:, ti - g0, :], stp_t[:, ti, :],
                                xr_sb[:, wi, :], start=True, stop=True)
                        nc.scalar.copy(z_sb[:, g0:g1, :], zps[:, :g1 - g0, :])
                    nc.vector.tensor_tensor(
                        out=ev[:, :nt, :], in0=z_sb[:, :nt, :],
                        in1=xj[:, :nt, :], op=ALU.add)
                    leaky(ev[:, :nt, :], ev[:, :nt, :], ATT_SLOPE, "lk")

                    alph = epool.tile([128, CHUNK_TILES, 2], F32, tag="alph")
                    prod = epool.tile([128, CHUNK_TILES, HID], BF16,
                                      tag="prod")
                    nc.vector.tensor_tensor(
                        out=prod[:, :nt, :], in0=ev[:, :nt, :],
                        in1=att_t[:].unsqueeze(1).to_broadcast([128, nt, HID]),
                        op=ALU.mult)
                    nc.vector.tensor_reduce(
                        out=alph[:, :nt, :],
                        in_=prod[:, :nt, :].rearrange("p t (h c) -> p t h c", h=2),
                        axis=mybir.AxisListType.X, op=ALU.add)

                    rhs = epool.tile([128, CHUNK_TILES, HID + 2], BF16, tag="rhs")
                    # exp(alpha) straight into the denominator columns (bf16)
                    nc.scalar.activation(rhs[:, :nt, HID:], alph[:, :nt, :],
                                         AF.Exp)
                    nc.vector.tensor_tensor(
                        out=rhs[:, :nt, :HID].rearrange("p t (h c) -> p t h c", h=2),
                        in0=xj[:, :nt, :].rearrange("p t (h c) -> p t h c", h=2),
                        in1=rhs[:, :nt, HID:].unsqueeze(3).to_broadcast(
                            [128, nt, 2, C]),
                        op=ALU.mult)

                    for t in range(t0, t1):
                        wi, part = tile_win[t], tile_part[t]
                        if cur is None or (cur[0], cur[1]) != (wi, part):
                            winps = wps.tile([WIN, HID + 2], F32, tag="winps",
                                             name=f"winps_{li}_{wi}_{part}")
                            cur = (wi, part, winps)
                        first = (t == 0) or (tile_win[t - 1], tile_part[t - 1]) != (wi, part)
                        last = (t == t_tot - 1) or \
                            (tile_win[t + 1], tile_part[t + 1]) != (wi, part)
                        nc.tensor.matmul(cur[2][:], s_t[:, t - t0, :],
                                         rhs[:, t - t0, :],
                                         start=first, stop=last)
                        if last:
                            dst = acc_t[:, wi, :]
                            nc.vector.tensor_tensor(out=dst, in0=dst,
                                                    in1=cur[2][:], op=ALU.add)
                            cur = None
                        if hooks and t in hooks:
                            hooks[t]()

            h2_t = npool.tile([128, NTN, HID], F32, tag="h2")
            # window -> last tile index (for transition hooks)
            last_tile = {}
            for t, wi in enumerate(tile_win):
                last_tile[wi] = t
            hook1 = max(last_tile[w] for w in range(min(24, WINS)))
            hook2 = max(last_tile[w] for w in range(min(40, WINS)))
            hooks = {}
            hooks2 = {}
            if WINS > 40 and hook2 > hook1:
                hooks = {hook1: (lambda: l1_post(0, 24)),
                         hook2: (lambda: l1_post(24, 40))}
                rest = 40
                hooks2 = {hook1: (lambda: finalize(1, 0, 24, h2_t, b2_t,
                                                   False, h1_t)),
                          hook2: (lambda: finalize(1, 24, 40, h2_t, b2_t,
                                                   False, h1_t))}
            else:
                rest = 0

            # ---------------- layer 1 ----------------
            xr1_t = npool.tile([128, NTN, HID], BF16, tag="xrsb1")
            nc.sync.dma_start(xr1_t[:].rearrange("p t f -> p (t f)"),
                              xr1_d[:])
            layer(0, att1_t, b1_t, h1_t, xr1_t, xl1_d, hooks)
            l1_post(rest, WINS)
            nc.gpsimd.collective_compute(
                "AllGather", mybir.AluOpType.bypass,
                replica_groups=[list(range(NCORES))],
                ins=[cin.opt()], outs=[xl_full.opt()])

            # ---------------- layer 2 ----------------
            layer(1, att2_t, b2_t, h2_t, xr_sb2, xl_full, hooks2)
            finalize(1, rest, WINS, h2_t, b2_t, False, h1_t)
            nc.sync.dma_start(out_d[:], h2_t[:])

    nc.compile()
    return nc


def make_inputs(x, edge_index, w_l1, w_r1, att1, b1, w_l2, w_r2, att2, b2):
    """Host-side prep: returns (meta, in_maps)."""
    meta, per_core = prep_edges(edge_index)
    x = np.asarray(x, dtype=np.float32)
    ident = np.eye(128, dtype=np.float32).astype(BF)

    def wcat(wl, wr):
        return np.concatenate([np.asarray(wl).T, np.asarray(wr).T],
                              axis=1).astype(BF)

    att_bc = lambda a: np.tile(np.asarray(a).reshape(1, HID), (128, 1)).astype(BF)
    b_bc = lambda b: np.tile(np.asarray(b).reshape(1, HID),
                             (128, 1)).astype(np.float32)

    w2 = wcat(w_l2, w_r2)
    a1, a2 = att_bc(att1), att_bc(att2)
    bb1, bb2 = b_bc(b1), b_bc(b2)

    # host-side layer-1 node transforms (mirrors device bf16 rounding:
    # bf16 inputs, f32 accumulate, bf16 store)
    xl1 = np.zeros((NCORES, NPC_PAD, HID), dtype=np.float32)
    xr1 = np.zeros((NCORES, NPC_PAD, HID), dtype=np.float32)
    xbf = x.astype(BF).astype(np.float32)
    wl1 = np.asarray(w_l1, dtype=np.float32).T.astype(BF).astype(np.float32)
    wr1 = np.asarray(w_r1, dtype=np.float32).T.astype(BF).astype(np.float32)
    for k in range(NCORES):
        xs = np.zeros((NPC_PAD, HID), dtype=np.float32)
        xs[:NPC] = xbf[k * NPC:(k + 1) * NPC]
        xl1[k] = xs @ wl1
        xr1[k] = xs @ wr1
    # p-major within each core slice: row (loc%128)*NTN + loc//128
    xl1_pm = xl1.reshape(NCORES, NTN, 128, HID).transpose(0, 2, 1, 3)
    xl1_tab = np.ascontiguousarray(
        xl1_pm.reshape(NP_GLOB, HID)).astype(BF)
    xr1_pm = xr1.reshape(NCORES, NTN, 128, HID).transpose(0, 2, 1, 3)

    in_maps = []
    for k in range(NCORES):
        in_maps.append({
            "xl1tab": xl1_tab,
            "xr1sb": np.ascontiguousarray(
                xr1_pm[k].reshape(128, NTN * HID)).astype(BF),
            "w2T": w2, "attbc1": a1, "attbc2": a2,
            "bias1": bb1, "bias2": bb2, "ident": ident,
            **per_core[k],
        })
    return meta, in_maps


def kernel(**inputs):
    from concourse.bass_utils import run_bass_kernel_spmd

    meta, in_maps = make_inputs(**inputs)
    nc = build_bass(meta)
    res = run_bass_kernel_spmd(nc, in_maps, list(range(NCORES)))
    outs = []
    for k in range(NCORES):
        o = res.results[k]["out"]          # [128, NTN, HID]
        outs.append(o.transpose(1, 0, 2).reshape(NPC_PAD, HID)[:NPC])
    return np.concatenate(outs, axis=0).astype(np.float32)

